# revision 15
# baseline (speedup 1.0000x reference)
"""Trainium2 Bass kernel for CRF Viterbi decode (nn_CRF_19353122636065).

Full inputs: emissions [128, 2048, 128] f32, transitions [128, 128] f32.
Output: (best_final_score [128] f32, best_final_label [128] int32).

Sequence-parallel strategy: the max-plus recurrence is run in the exp domain
(log-sum-exp at beta=12 ~ max), where a step is one [128x128]x[128,128] bf16
TensorE matmul plus one DVE multiply by the emission factor X_t. Viterbi
paths coalesce within ~32-48 steps (measured), so time is split into S=24
segments of P=82 payload steps, each preceded by an H=40-step halo warm-up
from a neutral state. Each core runs 3 independent segment chains at full
batch width; serial chain length drops from 2000 steps to 122.

Each chain records its state at the anchor (end of halo) and at its end.
Under coalescence, two runs' states at the same time differ by a per-batch
constant, so the host telescopes segment constants from the exact host
warm-up (T0=79 steps) to reconstruct absolute scores; argmax labels come
from the last segment directly.

Range management: X is max-centered per (b,t) on the host (bf16), a renorm
samples the per-batch partition max every R=8 steps (GPSIMD) and folds
fac^(1/4) into the next 4 X slices (DVE 4x-mode broadcast multiply, off the
critical path); the per-step multiply is a fused (min, mult)
scalar_tensor_tensor that clamps the PSUM operand at e^70 so overflow is
impossible. All applied factors are stashed and unwound exactly on the host.
"""

import numpy as np
import sys
from contextlib import ExitStack

sys.path.insert(0, "/opt/trn_rl_repo")

import concourse.bass as bass
import concourse.bacc as bacc
import concourse.tile as tile
from concourse import mybir
from concourse import bass_utils
from concourse import bass_isa

import ml_dtypes

B, T, L = 128, 2048, 128
START_LABEL, STOP_LABEL = 126, 127
NCORE = 8

BETA = 12.0
T_OFF = 2.5          # transition offset folded into W
BIAS_STEP = 0.0      # extra per-step down-shift in X (clamp handles up-tails)
S = 24               # segments
K = S // NCORE       # chains per core
H = 32               # halo steps
T0 = 2047 - ((2047 - 79) // S) * S   # host-exact warmup steps (=79)
P = (2047 - T0) // S                 # payload steps per segment (=82)
N = H + P                            # device steps per chain (=122)
R = 8                # renorm sample period
FOLD_OFF = 4         # sample at r applies to X slices [r+4, r+7]
TGT = -7.0           # renorm target: sum_k u -> e^TGT
CLAMP = float(np.exp(70.0))
U0VAL = float(np.exp(-10.0))
RLIST = [3] + [r for r in range(R - 1, N, R) if r + FOLD_OFF <= N - 1]
NREN = len(RLIST)

F32 = mybir.dt.float32
BF16 = mybir.dt.bfloat16

_BUILT = None


def _build_module():
    nc = bacc.Bacc(
        "TRN2",
        target_bir_lowering=False,
        debug=False,
        enable_asserts=False,
        num_devices=NCORE,
    )
    w_d = nc.dram_tensor("wmat", [L, L], F32, kind="ExternalInput")
    x_d = nc.dram_tensor("xin", [L, K * N, B], BF16, kind="ExternalInput")
    h_d = nc.dram_tensor("houts", [L, K, B], F32, kind="ExternalOutput")
    wo_d = nc.dram_tensor("wouts", [L, K, B], F32, kind="ExternalOutput")
    st_d = nc.dram_tensor("stash", [1, K * NREN, B], BF16, kind="ExternalOutput")

    CHUNKS = [8, 32, 32, N - 72]  # X chunk sizes; small first so compute starts fast
    CB = [0]
    for sz in CHUNKS:
        CB.append(CB[-1] + sz)

    with tile.TileContext(nc) as tc:
        with ExitStack() as ctx:
            singles = ctx.enter_context(tc.tile_pool(name="singles", bufs=1))
            upool = ctx.enter_context(tc.tile_pool(name="upool", bufs=1))
            xpool = ctx.enter_context(tc.tile_pool(name="xpool", bufs=1))
            rpool = ctx.enter_context(tc.tile_pool(name="rpool", bufs=4))
            psumP = ctx.enter_context(tc.tile_pool(name="psumP", bufs=2, space="PSUM"))
            psumR = ctx.enter_context(tc.tile_pool(name="psumR", bufs=1, space="PSUM"))

            wf = singles.tile([L, L], F32)
            nc.sync.dma_start(out=wf, in_=w_d.ap())
            wb = singles.tile([L, L], BF16)
            nc.scalar.copy(wb, wf)
            ones_col = singles.tile([L, 1], BF16)
            nc.vector.memset(ones_col, 1.0)
            ones_row = singles.tile([1, L], BF16)
            nc.vector.memset(ones_row, 1.0)
            stash_sb = singles.tile([1, K * NREN, B], BF16)

            xt = {c: [] for c in range(K)}
            for c in range(K):
                for k, sz in enumerate(CHUNKS):
                    xt[c].append(xpool.tile([L, sz, B], BF16, name=f"xc{c}_{k}"))
            dma_eng = [nc.sync, nc.scalar]
            for k in range(len(CHUNKS)):
                for c in range(K):
                    eng = dma_eng[(k * K + c) % 2]
                    eng.dma_start(
                        out=xt[c][k],
                        in_=x_d.ap()[:, c * N + CB[k]:c * N + CB[k + 1], :])

            def xslice(c, lo, hi):
                # contiguous slice [lo, hi) of chain c's X; returns list of APs
                out = []
                for k in range(len(CHUNKS)):
                    s, e = max(lo, CB[k]), min(hi, CB[k + 1])
                    if s < e:
                        out.append(xt[c][k][:, s - CB[k]:e - CB[k], :])
                return out

            u_tiles = {c: [upool.tile([L, B], BF16, name=f"u{c}_{k}") for k in range(6)]
                       for c in range(K)}
            for c in range(K):
                nc.vector.memset(u_tiles[c][0], U0VAL)

            ridx = 0
            for i in range(N):
                for c in range(K):
                    p = psumP.tile([L, B], F32, tag=f"P{c}")
                    nc.tensor.matmul(p, wb, u_tiles[c][i % 6], start=True, stop=True)
                    u_new = u_tiles[c][(i + 1) % 6]
                    xs = xslice(c, i, i + 1)[0]
                    # u_new = min(y, CLAMP) * X  (one DVE op, overflow-proof)
                    nc.vector.scalar_tensor_tensor(
                        out=u_new, in0=p, scalar=CLAMP, in1=xs.squeeze(1),
                        op0=mybir.AluOpType.min, op1=mybir.AluOpType.mult,
                    )
                    if i in RLIST:
                        # colsum via PE: ssum[0,b] = sum_k u_new[k,b]
                        ssum = psumR.tile([1, B], F32, tag="S", name=f"ssum{c}_{i}")
                        nc.tensor.matmul(ssum, ones_col, u_new, start=True, stop=True)
                        ssb = rpool.tile([1, B], F32, tag="ssb", name=f"ssb{c}_{i}")
                        # ssb = ssum + 1e-35: zero-sum guard so 1/ssb stays finite
                        nc.scalar.activation(out=ssb, in_=ssum[0:1, :],
                                             func=mybir.ActivationFunctionType.Copy,
                                             bias=1e-35)
                        inv = rpool.tile([1, B], F32, tag="inv", name=f"inv{c}_{i}")
                        nc.vector.reciprocal(inv, ssb)
                        s1 = rpool.tile([1, B], F32, tag="s1", name=f"s1{c}_{i}")
                        nc.scalar.activation(out=s1, in_=inv,
                                             func=mybir.ActivationFunctionType.Sqrt)
                        rf4 = rpool.tile([1, B], BF16, tag="rf4", name=f"rf4{c}_{i}")
                        # rf4 = sqrt(e^{TGT/2} * s1) = e^{TGT/4} * ssum^{-1/4}
                        nc.scalar.activation(out=rf4, in_=s1,
                                             func=mybir.ActivationFunctionType.Sqrt,
                                             scale=float(np.exp(TGT / 2.0)))
                        # broadcast across partitions via PE outer product
                        pbc = psumR.tile([L, B], F32, tag="Bc", name=f"pbc{c}_{i}")
                        nc.tensor.matmul(pbc, ones_row, rf4, start=True, stop=True)
                        rf4f = rpool.tile([L, B], BF16, tag="rf4f", name=f"rf4f{c}_{i}")
                        nc.scalar.copy(rf4f, pbc)
                        for sl in xslice(c, i + FOLD_OFF, i + FOLD_OFF + 4):
                            nsl = sl.shape[1]
                            rb = rf4f.unsqueeze(1).broadcast_to([L, nsl, B])
                            nc.vector.tensor_mul(sl, sl, rb)
                        j = c * NREN + RLIST.index(i)
                        nc.scalar.copy(stash_sb[:, j, :], rf4[0:1, :])
                if i == H - 1:
                    hcap = singles.tile([L, K, B], F32)
                    for c in range(K):
                        nc.scalar.copy(hcap[:, c, :], u_tiles[c][H % 6])
            wcap = singles.tile([L, K, B], F32)
            for c in range(K):
                nc.scalar.copy(wcap[:, c, :], u_tiles[c][N % 6])
            nc.sync.dma_start(out=h_d.ap(), in_=hcap)
            nc.sync.dma_start(out=wo_d.ap(), in_=wcap)
            nc.sync.dma_start(out=st_d.ap(), in_=stash_sb)

    nc.compile()
    return nc


def _exact_steps(v, tr, em_t):
    return (v[:, :, None] + tr[None, :, :]).max(axis=1) + em_t


def _bf16_round(x):
    x = np.ascontiguousarray(x, np.float32)
    u = x.view(np.uint32)
    return (((u.astype(np.uint64) + 0x7FFF + ((u >> 16) & 1)) >> 16)
            .astype(np.uint16))


def kernel(emissions: np.ndarray, transitions: np.ndarray):
    global _BUILT
    em = np.ascontiguousarray(np.asarray(emissions, dtype=np.float32))
    tr = np.ascontiguousarray(np.asarray(transitions, dtype=np.float32))
    assert em.shape == (B, T, L) and tr.shape == (L, L)

    # ---- host: exact warmup to T0 + c0 calibration ----
    rowmax = em.max(axis=2)  # [B,T]
    v = np.full((B, L), -10000.0, dtype=np.float32)
    v[:, START_LABEL] = 0.0
    incs = []
    for t in range(1, T0 + 1):
        vn = _exact_steps(v, tr, em[:, t, :])
        if t > 40:
            incs.append(float((vn.max(axis=1) - v.max(axis=1)
                               - rowmax[:, t]).mean()))
        v = vn
    v_T0 = v.astype(np.float64)
    c0 = float(np.mean(incs))
    shift = rowmax + (c0 + BIAS_STEP - T_OFF)  # [B,T]; T_OFF lives in W

    if _BUILT is None:
        _BUILT = (_build_module(),)
    nc = _BUILT[0]

    # ---- X = bf16(exp(beta*(em - shift))), layout [L, T, B] ----
    Xf = np.exp(BETA * (em - shift[:, :, None]), dtype=np.float32)
    Xu = _bf16_round(Xf)                        # [B,T,L] uint16
    X_LTB = np.ascontiguousarray(Xu.transpose(2, 1, 0))   # [L,T,B]

    wmat = _bf16_round(np.exp(BETA * (tr.astype(np.float64) - T_OFF))
                       .astype(np.float32)).view(ml_dtypes.bfloat16)
    wmat_f32 = wmat.astype(np.float32)

    e_pts = [T0 + s * P for s in range(S + 1)]  # segment boundaries
    in_maps = []
    for core in range(NCORE):
        xin = np.empty((L, K * N, B), np.uint16)
        for c in range(K):
            seg = core * K + c + 1              # 1-based segment id
            a = e_pts[seg - 1] - H              # neutral start time
            xin[:, c * N:(c + 1) * N, :] = X_LTB[:, a + 1:e_pts[seg] + 1, :]
        in_maps.append({
            "wmat": wmat_f32,
            "xin": xin.view(ml_dtypes.bfloat16),
        })

    res = bass_utils.run_bass_kernel_spmd(nc, in_maps, core_ids=list(range(NCORE)))

    # ---- host: telescoping reconstruction in f64 ----
    ln_rf = {}
    hl = {}
    wl = {}
    for core in range(NCORE):
        out = res.results[core]
        hq = np.asarray(out["houts"], dtype=np.float64)   # [L,K,B]
        wq = np.asarray(out["wouts"], dtype=np.float64)
        stq = np.asarray(out["stash"], dtype=np.float64).reshape(K * NREN, B)
        for c in range(K):
            seg = core * K + c + 1
            hl[seg] = np.log(np.maximum(hq[:, c, :], 1e-300)).T   # [B,L]
            wl[seg] = np.log(np.maximum(wq[:, c, :], 1e-300)).T
            ln_rf[seg] = np.log(np.maximum(stq[c * NREN:(c + 1) * NREN, :],
                                           1e-300))               # [NREN,B]

    cnt_h = np.array([min(max(H - (r + FOLD_OFF), 0), 4) for r in RLIST], np.float64)
    cnt_w = np.array([min(max(N - (r + FOLD_OFF), 0), 4) for r in RLIST], np.float64)

    Vabs = v_T0                                  # [B,L] absolute at e_0 = T0
    for seg in range(1, S + 1):
        a = e_pts[seg - 1] - H
        lf_h = (cnt_h[:, None] * ln_rf[seg]).sum(axis=0)   # [B]
        lf_w = (cnt_w[:, None] * ln_rf[seg]).sum(axis=0)
        sh_h = shift[:, a + 1:a + H + 1].sum(axis=1).astype(np.float64)
        sh_w = shift[:, a + 1:e_pts[seg] + 1].sum(axis=1).astype(np.float64)
        vh = (hl[seg] - lf_h[:, None]) / BETA + sh_h[:, None] + H * T_OFF
        vw = (wl[seg] - lf_w[:, None]) / BETA + sh_w[:, None] + N * T_OFF
        jstar = Vabs.argmax(axis=1)
        d = Vabs[np.arange(B), jstar] - vh[np.arange(B), jstar]
        Vabs = vw + d[:, None]

    vT = Vabs + tr[:, STOP_LABEL].astype(np.float64)[None, :]
    scores = vT.max(axis=1).astype(np.float32)
    labels = vT.argmax(axis=1).astype(np.int32)
    return scores, labels


if __name__ == "__main__":
    rng = np.random.default_rng(0)
    em = rng.standard_normal((B, T, L)).astype(np.float32)
    tr = rng.standard_normal((L, L)).astype(np.float32)
    tr[:, START_LABEL] = 0.0
    tr[STOP_LABEL, :] = 0.0
    s, l = kernel(em, tr)
    print(s[:8], l[:8])


# revision 16
# speedup vs baseline: 1.0487x; 1.0487x over previous
"""Trainium2 Bass kernel for CRF Viterbi decode (nn_CRF_19353122636065).

Full inputs: emissions [128, 2048, 128] f32, transitions [128, 128] f32.
Output: (best_final_score [128] f32, best_final_label [128] int32).

Sequence-parallel strategy: the max-plus recurrence is run in the exp domain
(log-sum-exp at beta=12 ~ max), where a step is one [128x128]x[128,128] bf16
TensorE matmul plus one DVE multiply by the emission factor X_t. Viterbi
paths coalesce within ~32-48 steps (measured), so time is split into S=24
segments of P=82 payload steps, each preceded by an H=40-step halo warm-up
from a neutral state. Each core runs 3 independent segment chains at full
batch width; serial chain length drops from 2000 steps to 122.

Each chain records its state at the anchor (end of halo) and at its end.
Under coalescence, two runs' states at the same time differ by a per-batch
constant, so the host telescopes segment constants from the exact host
warm-up (T0=79 steps) to reconstruct absolute scores; argmax labels come
from the last segment directly.

Range management: X is max-centered per (b,t) on the host (bf16), a renorm
samples the per-batch partition max every R=8 steps (GPSIMD) and folds
fac^(1/4) into the next 4 X slices (DVE 4x-mode broadcast multiply, off the
critical path); the per-step multiply is a fused (min, mult)
scalar_tensor_tensor that clamps the PSUM operand at e^70 so overflow is
impossible. All applied factors are stashed and unwound exactly on the host.
"""

import numpy as np
import sys
from contextlib import ExitStack

sys.path.insert(0, "/opt/trn_rl_repo")

import concourse.bass as bass
import concourse.bacc as bacc
import concourse.tile as tile
from concourse import mybir
from concourse import bass_utils
from concourse import bass_isa

import ml_dtypes

B, T, L = 128, 2048, 128
START_LABEL, STOP_LABEL = 126, 127
NCORE = 8

BETA = 12.0
T_OFF = 2.5          # transition offset folded into W
BIAS_STEP = 0.0      # extra per-step down-shift in X (clamp handles up-tails)
S = 24               # segments
K = S // NCORE       # chains per core
H = 32               # halo steps
T0 = 2047 - ((2047 - 79) // S) * S   # host-exact warmup steps (=79)
P = (2047 - T0) // S                 # payload steps per segment (=82)
N = H + P                            # device steps per chain (=122)
R = 8                # renorm sample period
FOLD_OFF = 4         # sample at r applies to X slices [r+4, r+7]
TGT = -7.0           # renorm target: sum_k u -> e^TGT
CLAMP = float(np.exp(70.0))
U0VAL = float(np.exp(-10.0))
RLIST = [3] + [r for r in range(R - 1, N, R) if r + FOLD_OFF <= N - 1]
NREN = len(RLIST)

F32 = mybir.dt.float32
BF16 = mybir.dt.bfloat16

_BUILT = None


def _build_module():
    nc = bacc.Bacc(
        "TRN2",
        target_bir_lowering=False,
        debug=False,
        enable_asserts=False,
        num_devices=NCORE,
    )
    w_d = nc.dram_tensor("wmat", [L, L], F32, kind="ExternalInput")
    x_d = nc.dram_tensor("xin", [L, K * N, B], BF16, kind="ExternalInput")
    h_d = nc.dram_tensor("houts", [L, K, B], F32, kind="ExternalOutput")
    wo_d = nc.dram_tensor("wouts", [L, K, B], F32, kind="ExternalOutput")
    st_d = nc.dram_tensor("stash", [1, K * NREN, B], BF16, kind="ExternalOutput")

    CHUNKS = [8, 32, 32, N - 72]  # X chunk sizes; small first so compute starts fast
    CB = [0]
    for sz in CHUNKS:
        CB.append(CB[-1] + sz)

    with tile.TileContext(nc) as tc:
        with ExitStack() as ctx:
            singles = ctx.enter_context(tc.tile_pool(name="singles", bufs=1))
            upool = ctx.enter_context(tc.tile_pool(name="upool", bufs=1))
            xpool = ctx.enter_context(tc.tile_pool(name="xpool", bufs=1))
            rpool = ctx.enter_context(tc.tile_pool(name="rpool", bufs=4))
            psumP = ctx.enter_context(tc.tile_pool(name="psumP", bufs=2, space="PSUM"))
            psumR = ctx.enter_context(tc.tile_pool(name="psumR", bufs=1, space="PSUM"))

            wf = singles.tile([L, L], F32)
            nc.sync.dma_start(out=wf, in_=w_d.ap())
            wb = singles.tile([L, L], BF16)
            nc.scalar.copy(wb, wf)
            ones_col = singles.tile([L, 1], BF16)
            nc.vector.memset(ones_col, 1.0)
            ones_row = singles.tile([1, L], BF16)
            nc.vector.memset(ones_row, 1.0)
            stash_sb = singles.tile([1, K * NREN, B], BF16)

            xt = {c: [] for c in range(K)}
            for c in range(K):
                for k, sz in enumerate(CHUNKS):
                    xt[c].append(xpool.tile([L, sz, B], BF16, name=f"xc{c}_{k}"))
            dma_eng = [nc.sync, nc.scalar]
            for k in range(len(CHUNKS)):
                for c in range(K):
                    eng = dma_eng[(k * K + c) % 2]
                    eng.dma_start(
                        out=xt[c][k],
                        in_=x_d.ap()[:, c * N + CB[k]:c * N + CB[k + 1], :])

            def xslice(c, lo, hi):
                # contiguous slice [lo, hi) of chain c's X; returns list of APs
                out = []
                for k in range(len(CHUNKS)):
                    s, e = max(lo, CB[k]), min(hi, CB[k + 1])
                    if s < e:
                        out.append(xt[c][k][:, s - CB[k]:e - CB[k], :])
                return out

            u_tiles = {c: [upool.tile([L, B], BF16, name=f"u{c}_{k}") for k in range(6)]
                       for c in range(K)}
            for c in range(K):
                nc.vector.memset(u_tiles[c][0], U0VAL)

            ridx = 0
            for i in range(N):
                for c in range(K):
                    p = psumP.tile([L, B], F32, tag=f"P{c}")
                    nc.tensor.matmul(p, wb, u_tiles[c][i % 6], start=True, stop=True)
                    u_new = u_tiles[c][(i + 1) % 6]
                    xs = xslice(c, i, i + 1)[0]
                    # u_new = min(y, CLAMP) * X  (one DVE op, overflow-proof)
                    nc.vector.scalar_tensor_tensor(
                        out=u_new, in0=p, scalar=CLAMP, in1=xs.squeeze(1),
                        op0=mybir.AluOpType.min, op1=mybir.AluOpType.mult,
                    )
                    if i in RLIST:
                        # colsum via PE: ssum[0,b] = sum_k u_new[k,b]
                        ssum = psumR.tile([1, B], F32, tag="S", name=f"ssum{c}_{i}")
                        nc.tensor.matmul(ssum, ones_col, u_new, start=True, stop=True)
                        inv = rpool.tile([1, B], F32, tag="inv", name=f"inv{c}_{i}")
                        nc.vector.reciprocal(inv, ssum[0:1, :])
                        s1 = rpool.tile([1, B], F32, tag="s1", name=f"s1{c}_{i}")
                        nc.scalar.activation(out=s1, in_=inv,
                                             func=mybir.ActivationFunctionType.Sqrt)
                        rf4 = rpool.tile([1, B], BF16, tag="rf4", name=f"rf4{c}_{i}")
                        # rf4 = sqrt(e^{TGT/2} * s1) = e^{TGT/4} * ssum^{-1/4}
                        nc.scalar.activation(out=rf4, in_=s1,
                                             func=mybir.ActivationFunctionType.Sqrt,
                                             scale=float(np.exp(TGT / 2.0)))
                        # broadcast across partitions via PE outer product
                        pbc = psumR.tile([L, B], F32, tag="Bc", name=f"pbc{c}_{i}")
                        nc.tensor.matmul(pbc, ones_row, rf4, start=True, stop=True)
                        rf4f = rpool.tile([L, B], BF16, tag="rf4f", name=f"rf4f{c}_{i}")
                        nc.scalar.copy(rf4f, pbc)
                        for sl in xslice(c, i + FOLD_OFF, i + FOLD_OFF + 4):
                            nsl = sl.shape[1]
                            rb = rf4f.unsqueeze(1).broadcast_to([L, nsl, B])
                            nc.vector.tensor_mul(sl, sl, rb)
                        j = c * NREN + RLIST.index(i)
                        nc.scalar.copy(stash_sb[:, j, :], rf4[0:1, :])
                if i == H - 1:
                    hcap = singles.tile([L, K, B], F32)
                    for c in range(K):
                        nc.scalar.copy(hcap[:, c, :], u_tiles[c][H % 6])
            wcap = singles.tile([L, K, B], F32)
            for c in range(K):
                nc.scalar.copy(wcap[:, c, :], u_tiles[c][N % 6])
            nc.sync.dma_start(out=h_d.ap(), in_=hcap)
            nc.sync.dma_start(out=wo_d.ap(), in_=wcap)
            nc.sync.dma_start(out=st_d.ap(), in_=stash_sb)

    nc.compile()
    return nc


def _exact_steps(v, tr, em_t):
    return (v[:, :, None] + tr[None, :, :]).max(axis=1) + em_t


def _bf16_round(x):
    x = np.ascontiguousarray(x, np.float32)
    u = x.view(np.uint32)
    return (((u.astype(np.uint64) + 0x7FFF + ((u >> 16) & 1)) >> 16)
            .astype(np.uint16))


def kernel(emissions: np.ndarray, transitions: np.ndarray):
    global _BUILT
    em = np.ascontiguousarray(np.asarray(emissions, dtype=np.float32))
    tr = np.ascontiguousarray(np.asarray(transitions, dtype=np.float32))
    assert em.shape == (B, T, L) and tr.shape == (L, L)

    # ---- host: exact warmup to T0 + c0 calibration ----
    rowmax = em.max(axis=2)  # [B,T]
    v = np.full((B, L), -10000.0, dtype=np.float32)
    v[:, START_LABEL] = 0.0
    incs = []
    for t in range(1, T0 + 1):
        vn = _exact_steps(v, tr, em[:, t, :])
        if t > 40:
            incs.append(float((vn.max(axis=1) - v.max(axis=1)
                               - rowmax[:, t]).mean()))
        v = vn
    v_T0 = v.astype(np.float64)
    c0 = float(np.mean(incs))
    shift = rowmax + (c0 + BIAS_STEP - T_OFF)  # [B,T]; T_OFF lives in W

    if _BUILT is None:
        _BUILT = (_build_module(),)
    nc = _BUILT[0]

    # ---- X = bf16(exp(beta*(em - shift))), layout [L, T, B] ----
    Xf = np.exp(BETA * (em - shift[:, :, None]), dtype=np.float32)
    Xu = _bf16_round(Xf)                        # [B,T,L] uint16
    X_LTB = np.ascontiguousarray(Xu.transpose(2, 1, 0))   # [L,T,B]

    wmat = _bf16_round(np.exp(BETA * (tr.astype(np.float64) - T_OFF))
                       .astype(np.float32)).view(ml_dtypes.bfloat16)
    wmat_f32 = wmat.astype(np.float32)

    e_pts = [T0 + s * P for s in range(S + 1)]  # segment boundaries
    in_maps = []
    for core in range(NCORE):
        xin = np.empty((L, K * N, B), np.uint16)
        for c in range(K):
            seg = core * K + c + 1              # 1-based segment id
            a = e_pts[seg - 1] - H              # neutral start time
            xin[:, c * N:(c + 1) * N, :] = X_LTB[:, a + 1:e_pts[seg] + 1, :]
        in_maps.append({
            "wmat": wmat_f32,
            "xin": xin.view(ml_dtypes.bfloat16),
        })

    res = bass_utils.run_bass_kernel_spmd(nc, in_maps, core_ids=list(range(NCORE)))

    # ---- host: telescoping reconstruction in f64 ----
    ln_rf = {}
    hl = {}
    wl = {}
    for core in range(NCORE):
        out = res.results[core]
        hq = np.asarray(out["houts"], dtype=np.float64)   # [L,K,B]
        wq = np.asarray(out["wouts"], dtype=np.float64)
        stq = np.asarray(out["stash"], dtype=np.float64).reshape(K * NREN, B)
        for c in range(K):
            seg = core * K + c + 1
            hl[seg] = np.log(np.maximum(hq[:, c, :], 1e-300)).T   # [B,L]
            wl[seg] = np.log(np.maximum(wq[:, c, :], 1e-300)).T
            ln_rf[seg] = np.log(np.maximum(stq[c * NREN:(c + 1) * NREN, :],
                                           1e-300))               # [NREN,B]

    cnt_h = np.array([min(max(H - (r + FOLD_OFF), 0), 4) for r in RLIST], np.float64)
    cnt_w = np.array([min(max(N - (r + FOLD_OFF), 0), 4) for r in RLIST], np.float64)

    Vabs = v_T0                                  # [B,L] absolute at e_0 = T0
    for seg in range(1, S + 1):
        a = e_pts[seg - 1] - H
        lf_h = (cnt_h[:, None] * ln_rf[seg]).sum(axis=0)   # [B]
        lf_w = (cnt_w[:, None] * ln_rf[seg]).sum(axis=0)
        sh_h = shift[:, a + 1:a + H + 1].sum(axis=1).astype(np.float64)
        sh_w = shift[:, a + 1:e_pts[seg] + 1].sum(axis=1).astype(np.float64)
        vh = (hl[seg] - lf_h[:, None]) / BETA + sh_h[:, None] + H * T_OFF
        vw = (wl[seg] - lf_w[:, None]) / BETA + sh_w[:, None] + N * T_OFF
        jstar = Vabs.argmax(axis=1)
        d = Vabs[np.arange(B), jstar] - vh[np.arange(B), jstar]
        Vabs = vw + d[:, None]

    vT = Vabs + tr[:, STOP_LABEL].astype(np.float64)[None, :]
    scores = vT.max(axis=1).astype(np.float32)
    labels = vT.argmax(axis=1).astype(np.int32)
    return scores, labels


if __name__ == "__main__":
    rng = np.random.default_rng(0)
    em = rng.standard_normal((B, T, L)).astype(np.float32)
    tr = rng.standard_normal((L, L)).astype(np.float32)
    tr[:, START_LABEL] = 0.0
    tr[STOP_LABEL, :] = 0.0
    s, l = kernel(em, tr)
    print(s[:8], l[:8])
